# revision 8
# baseline (speedup 1.0000x reference)
"""Trainium2 Bass kernel for CustomMLP: out = GELU(x@W1+b1)@W2 + b2.

x: (4, 2048, 1024) f32, W1: (1024, 4096), b1: (4096,), W2: (4096, 1024),
b2: (1024,). Data-parallel over the 8192 flattened rows: each of the 8
NeuronCores handles 1024 rows with fully replicated weights (no
collectives).

Per-core layout (everything transposed so both matmuls contract on the
partition axis with no on-chip transposes):
  xT   [1024(e), 1024(m)]           = x_shard^T, bf16
  hT   [h, m] computed on chip      (GELU applied on PSUM eviction), bf16
  outT [1024(e2), 1024(m)]          f32; host transposes back

matmul1: psum[h_blk, m] += w1[e_blk, h_blk].T @ xT[e_blk, m]
matmul2: psum[e2_blk, m] += w2[h_blk, e2_blk].T @ hT[h_blk, m]

All matmul operands are bf16 (fp32 PSUM accumulation): same 1 cycle/row
PE rate as fp32r but FWL (fast weight load) hides LDWEIGHTS, and HBM
traffic halves.  Weights are host-packed so every DMA lands contiguous
per partition.  DMA is split across both HWDGE queues: w1 on the scalar
queue (batched 4 h-blocks per DMA), xT/w2/out on the sync queue.  A few
warm-up matmuls on memset tiles run during the head DMA wait so the HAM
clock-gate ramp (1.2->2.4 GHz) is absorbed before real work arrives.
"""
import ml_dtypes
import numpy as np

import concourse.bass as bass
import concourse.mybir as mybir
import concourse.tile as tile
from concourse import bacc
from concourse.bass_utils import run_bass_kernel_spmd

P = 128
N_CORES = 8

F32 = mybir.dt.float32
BF16 = mybir.dt.bfloat16
NP_BF16 = ml_dtypes.bfloat16
GELU = mybir.ActivationFunctionType.Gelu
IDENT = mybir.ActivationFunctionType.Identity


def build_nc(M=1024, E=1024, H=4096, E2=1024, mm_dtype=BF16, act=GELU, nwarm=6):
    """Build + compile the per-core program. M/E/H/E2 parameterized so a
    scaled-down version can run in CoreSim."""
    EB, HB, E2B = E // P, H // P, E2 // P
    MH = max(1, M // 512)  # m halves (moving-dim chunks of <=512)
    MS = M // MH           # moving chunk size
    HEAD = min(4, HB)      # singly-loaded head h-blocks
    W1G = min(4, HB - HEAD) or 1  # w1 h-blocks per grouped DMA

    mmdt = mm_dtype
    nc = bacc.Bacc(None, target_bir_lowering=False)
    xT_d = nc.declare_dram_parameter("xT", [E, M], mmdt, isOutput=False)
    w1_d = nc.declare_dram_parameter("w1p", [HB, P, EB, P], mmdt, isOutput=False)
    b1_d = nc.declare_dram_parameter("b1p", [P, HB], F32, isOutput=False)
    w2_d = nc.declare_dram_parameter("w2p", [E2B, P, HB, P], mmdt, isOutput=False)
    b2_d = nc.declare_dram_parameter("b2p", [P, E2B], F32, isOutput=False)
    out_d = nc.declare_dram_parameter("outT", [E2B, P, M], F32, isOutput=True)

    xT_v = xT_d.rearrange("(eb p) m -> p eb m", p=P)
    w1_v = w1_d.rearrange("hb p eb q -> p hb eb q")

    with tile.TileContext(nc) as tc:
        with (
            tc.tile_pool(name="const", bufs=1) as cpool,
            tc.tile_pool(name="xp", bufs=1) as xpool,
            tc.tile_pool(name="hp", bufs=1) as hpool,
            tc.tile_pool(name="w1h", bufs=1) as w1head,
            tc.tile_pool(name="w1p", bufs=3) as w1pool,
            tc.tile_pool(name="w2p", bufs=3) as w2pool,
            tc.tile_pool(name="op", bufs=2) as opool,
            tc.tile_pool(name="ps1", bufs=3, space="PSUM") as psum1,
            tc.tile_pool(name="ps2", bufs=3, space="PSUM") as psum2,
            tc.tile_pool(name="psw", bufs=1, space="PSUM") as psumw,
        ):
            # ---- HAM warm-up: dummy matmuls on memset tiles keep the PE
            # busy through the clock-gate ramp while the head DMAs land.
            if nwarm:
                wm_l = cpool.tile([P, P], mmdt, name="wml")
                wm_r = cpool.tile([P, MS], mmdt, name="wmr")
                nc.vector.memset(wm_l[:], 0.0)
                nc.vector.memset(wm_r[:], 0.0)
                ps_w = psumw.tile([P, MS], F32, name="psw")
                for _ in range(nwarm):
                    nc.tensor.matmul(
                        ps_w[:], lhsT=wm_l[:], rhs=wm_r[:], start=True, stop=True
                    )

            # ---- head DMAs, all on the sync HWDGE queue in consumption
            # order: w1 hb0 first, then xT eb0 so matmul 0 starts early,
            # then the bulk loads.
            w1_head = w1head.tile([P, HEAD, EB, P], mmdt, name="w1t")
            xT_sb = xpool.tile([P, EB, M], mmdt, name="xT")
            nc.sync.dma_start(out=w1_head[:, 0], in_=w1_d[0])
            nc.sync.dma_start(out=xT_sb[:, 0, 0:MS], in_=xT_v[:, 0, 0:MS])
            if EB > 1:
                nc.sync.dma_start(out=xT_sb[:, 1:EB, 0:MS], in_=xT_v[:, 1:EB, 0:MS])
            if HEAD > 1:
                nc.sync.dma_start(
                    out=w1_head[:, 1:HEAD], in_=w1_v[:, 1:HEAD]
                )
            b1_sb = cpool.tile([P, HB], F32, name="b1s")
            b2_sb = cpool.tile([P, E2B], F32, name="b2s")
            nc.sync.dma_start(out=b1_sb[:], in_=b1_d[:])
            nc.sync.dma_start(out=b2_sb[:], in_=b2_d[:])
            for mh in range(1, MH):
                ms = slice(mh * MS, (mh + 1) * MS)
                nc.sync.dma_start(out=xT_sb[:, :, ms], in_=xT_v[:, :, ms])

            hT_sb = hpool.tile([P, HB, M], mmdt, name="hT")

            def mm1_group(w1_t, hb, mh):
                ms = slice(mh * MS, (mh + 1) * MS)
                ps = psum1.tile([P, MS], F32, name="ps1")
                for eb in range(EB):
                    nc.tensor.matmul(
                        ps[:],
                        lhsT=w1_t[:, eb, :],
                        rhs=xT_sb[:, eb, ms],
                        start=(eb == 0),
                        stop=(eb == EB - 1),
                    )
                nc.scalar.activation(
                    hT_sb[:, hb, ms], ps[:], act, bias=b1_sb[:, hb : hb + 1]
                )

            # ---- matmul 1 + GELU ----
            # Head h-blocks run m-half-major so the PE's early xT demand
            # rate is halved while the DMA queue ramps.
            for mh in range(MH):
                for hb in range(HEAD):
                    mm1_group(w1_head[:, hb], hb, mh)
            for g0 in range(HEAD, HB, W1G):
                gn = min(W1G, HB - g0)
                w1_t = w1pool.tile([P, W1G, EB, P], mmdt, name="w1g")
                nc.sync.dma_start(
                    out=w1_t[:, 0:gn], in_=w1_v[:, g0 : g0 + gn]
                )
                for j in range(gn):
                    for mh in range(MH):
                        mm1_group(w1_t[:, j], g0 + j, mh)

            # ---- matmul 2 + bias ----
            for e2b in range(E2B):
                w2_t = w2pool.tile([P, HB, P], mmdt, name="w2t")
                nc.sync.dma_start(out=w2_t[:], in_=w2_d[e2b])
                out_sb = opool.tile([P, M], F32, name="outsb")
                for mh in range(MH):
                    ms = slice(mh * MS, (mh + 1) * MS)
                    ps2 = psum2.tile([P, MS], F32, name="ps2")
                    for hb in range(HB):
                        nc.tensor.matmul(
                            ps2[:],
                            lhsT=w2_t[:, hb, :],
                            rhs=hT_sb[:, hb, ms],
                            start=(hb == 0),
                            stop=(hb == HB - 1),
                        )
                    nc.scalar.activation(
                        out_sb[:, ms], ps2[:], IDENT, bias=b2_sb[:, e2b : e2b + 1]
                    )
                    nc.sync.dma_start(out=out_d[e2b, :, ms], in_=out_sb[:, ms])

    nc.compile()
    return nc


def pack_inputs(x, w1, b1, w2, b2):
    """Host-side shard + pack. Returns per-core input maps."""
    M_TOT = x.shape[0] * x.shape[1]
    E = x.shape[2]
    H = w1.shape[1]
    E2 = w2.shape[1]
    MC = M_TOT // N_CORES
    xf = np.ascontiguousarray(x.reshape(M_TOT, E))

    w1p = np.ascontiguousarray(
        w1.reshape(E // P, P, H // P, P).transpose(2, 1, 0, 3)
    ).astype(NP_BF16)
    w2p = np.ascontiguousarray(
        w2.reshape(H // P, P, E2 // P, P).transpose(2, 1, 0, 3)
    ).astype(NP_BF16)
    b1p = np.ascontiguousarray(b1.reshape(H // P, P).T)
    b2p = np.ascontiguousarray(b2.reshape(E2 // P, P).T)

    in_maps = []
    for i in range(N_CORES):
        xTi = np.ascontiguousarray(xf[i * MC : (i + 1) * MC].T).astype(NP_BF16)
        in_maps.append(
            {"xT": xTi, "w1p": w1p, "b1p": b1p, "w2p": w2p, "b2p": b2p}
        )
    return in_maps


def unpack_outputs(results, batch_shape=(4, 2048), E2=1024):
    M_TOT = batch_shape[0] * batch_shape[1]
    MC = M_TOT // N_CORES
    out = np.empty((M_TOT, E2), dtype=np.float32)
    for i in range(N_CORES):
        o = results[i]["outT"]  # [E2B, P, MC]
        out[i * MC : (i + 1) * MC] = o.transpose(2, 0, 1).reshape(MC, E2)
    return out.reshape(*batch_shape, E2)


_NC_CACHE = {}


def _get_nc():
    if "nc" not in _NC_CACHE:
        _NC_CACHE["nc"] = build_nc()
    return _NC_CACHE["nc"]


def kernel(x, w1, b1, w2, b2):
    nc = _get_nc()
    in_maps = pack_inputs(
        np.asarray(x, dtype=np.float32),
        np.asarray(w1, dtype=np.float32),
        np.asarray(b1, dtype=np.float32),
        np.asarray(w2, dtype=np.float32),
        np.asarray(b2, dtype=np.float32),
    )
    res = run_bass_kernel_spmd(nc, in_maps, core_ids=list(range(N_CORES))).results
    return unpack_outputs(res, batch_shape=(x.shape[0], x.shape[1]), E2=w2.shape[1])


# revision 10
# speedup vs baseline: 1.0047x; 1.0047x over previous
"""Trainium2 Bass kernel for CustomMLP: out = GELU(x@W1+b1)@W2 + b2.

x: (4, 2048, 1024) f32, W1: (1024, 4096), b1: (4096,), W2: (4096, 1024),
b2: (1024,). Data-parallel over the 8192 flattened rows: each of the 8
NeuronCores handles 1024 rows with fully replicated weights (no
collectives).

Per-core layout (everything transposed so both matmuls contract on the
partition axis with no on-chip transposes):
  xT   [1024(e), 1024(m)]           = x_shard^T, bf16
  hT   [h, m] computed on chip      (GELU applied on PSUM eviction), bf16
  outT [1024(e2), 1024(m)]          f32; host transposes back

matmul1: psum[h_blk, m] += w1[e_blk, h_blk].T @ xT[e_blk, m]
matmul2: psum[e2_blk, m] += w2[h_blk, e2_blk].T @ hT[h_blk, m]

All matmul operands are bf16 (fp32 PSUM accumulation): same 1 cycle/row
PE rate as fp32r but FWL (fast weight load) hides LDWEIGHTS, and HBM
traffic halves.  Weights are host-packed so every DMA lands contiguous
per partition.  DMA is split across both HWDGE queues: w1 on the scalar
queue (batched 4 h-blocks per DMA), xT/w2/out on the sync queue.  A few
warm-up matmuls on memset tiles run during the head DMA wait so the HAM
clock-gate ramp (1.2->2.4 GHz) is absorbed before real work arrives.
"""
import ml_dtypes
import numpy as np

import concourse.bass as bass
import concourse.mybir as mybir
import concourse.tile as tile
from concourse import bacc
from concourse.bass_utils import run_bass_kernel_spmd

P = 128
N_CORES = 8

F32 = mybir.dt.float32
BF16 = mybir.dt.bfloat16
NP_BF16 = ml_dtypes.bfloat16
GELU = mybir.ActivationFunctionType.Gelu
IDENT = mybir.ActivationFunctionType.Identity


def build_nc(M=1024, E=1024, H=4096, E2=1024, mm_dtype=BF16, act=GELU, nwarm=3):
    """Build + compile the per-core program. M/E/H/E2 parameterized so a
    scaled-down version can run in CoreSim."""
    EB, HB, E2B = E // P, H // P, E2 // P
    MH = max(1, M // 512)  # m halves (moving-dim chunks of <=512)
    MS = M // MH           # moving chunk size
    HEAD = min(4, HB)      # singly-loaded head h-blocks
    W1G = min(4, HB - HEAD) or 1  # w1 h-blocks per grouped DMA

    mmdt = mm_dtype
    nc = bacc.Bacc(None, target_bir_lowering=False)
    xT_d = nc.declare_dram_parameter("xT", [E, M], mmdt, isOutput=False)
    w1_d = nc.declare_dram_parameter("w1p", [HB, P, EB, P], mmdt, isOutput=False)
    b1_d = nc.declare_dram_parameter("b1p", [P, HB], F32, isOutput=False)
    w2_d = nc.declare_dram_parameter("w2p", [E2B, P, HB, P], mmdt, isOutput=False)
    b2_d = nc.declare_dram_parameter("b2p", [P, E2B], F32, isOutput=False)
    out_d = nc.declare_dram_parameter("outT", [E2B, P, M], F32, isOutput=True)

    xT_v = xT_d.rearrange("(eb p) m -> p eb m", p=P)
    w1_v = w1_d.rearrange("hb p eb q -> p hb eb q")

    with tile.TileContext(nc) as tc:
        with (
            tc.tile_pool(name="const", bufs=1) as cpool,
            tc.tile_pool(name="xp", bufs=1) as xpool,
            tc.tile_pool(name="hp", bufs=1) as hpool,
            tc.tile_pool(name="w1h", bufs=1) as w1head,
            tc.tile_pool(name="w1p", bufs=3) as w1pool,
            tc.tile_pool(name="w2p", bufs=3) as w2pool,
            tc.tile_pool(name="op", bufs=2) as opool,
            tc.tile_pool(name="ps1", bufs=3, space="PSUM") as psum1,
            tc.tile_pool(name="ps2", bufs=3, space="PSUM") as psum2,
            tc.tile_pool(name="psw", bufs=1, space="PSUM") as psumw,
        ):
            # ---- HAM warm-up: dummy matmuls on memset tiles keep the PE
            # busy through the clock-gate ramp while the head DMAs land.
            if nwarm:
                wm_l = cpool.tile([P, P], mmdt, name="wml")
                wm_r = cpool.tile([P, MS], mmdt, name="wmr")
                nc.vector.memset(wm_l[:], 0.0)
                nc.vector.memset(wm_r[:], 0.0)
                ps_w = psumw.tile([P, MS], F32, name="psw")
                for _ in range(nwarm):
                    nc.tensor.matmul(
                        ps_w[:], lhsT=wm_l[:], rhs=wm_r[:], start=True, stop=True
                    )

            # ---- head DMAs, all on the sync HWDGE queue in consumption
            # order: w1 hb0 first, then xT eb0 so matmul 0 starts early,
            # then the bulk loads.
            # Interleave ~256KB xT chunks with single w1 head blocks so
            # early PE demand (eb-chunk pacing in group 0, then one w1
            # block per 1.7us group) tracks the ~358 GB/s supply with
            # fine-grained completion semaphores.
            w1_head = w1head.tile([P, HEAD, EB, P], mmdt, name="w1t")
            xT_sb = xpool.tile([P, EB, M], mmdt, name="xT")
            nc.sync.dma_start(out=w1_head[:, 0], in_=w1_d[0])
            XC = 2  # xT eb-blocks per head DMA
            xt_chunks = [
                (c0, min(c0 + XC, EB)) for c0 in range(0, EB, XC)
            ]
            nhead = max(len(xt_chunks), HEAD - 1)
            for k in range(nhead):
                if k < len(xt_chunks):
                    c0, c1 = xt_chunks[k]
                    nc.sync.dma_start(
                        out=xT_sb[:, c0:c1, 0:MS], in_=xT_v[:, c0:c1, 0:MS]
                    )
                if 1 + k < HEAD:
                    nc.sync.dma_start(
                        out=w1_head[:, 1 + k], in_=w1_d[1 + k]
                    )
            b1_sb = cpool.tile([P, HB], F32, name="b1s")
            b2_sb = cpool.tile([P, E2B], F32, name="b2s")
            nc.sync.dma_start(out=b1_sb[:], in_=b1_d[:])
            nc.sync.dma_start(out=b2_sb[:], in_=b2_d[:])
            for mh in range(1, MH):
                ms = slice(mh * MS, (mh + 1) * MS)
                half = EB // 2 or EB
                for c0 in range(0, EB, half):
                    c1 = min(c0 + half, EB)
                    nc.sync.dma_start(
                        out=xT_sb[:, c0:c1, ms], in_=xT_v[:, c0:c1, ms]
                    )

            hT_sb = hpool.tile([P, HB, M], mmdt, name="hT")

            def mm1_group(w1_t, hb, mh):
                ms = slice(mh * MS, (mh + 1) * MS)
                ps = psum1.tile([P, MS], F32, name="ps1")
                for eb in range(EB):
                    nc.tensor.matmul(
                        ps[:],
                        lhsT=w1_t[:, eb, :],
                        rhs=xT_sb[:, eb, ms],
                        start=(eb == 0),
                        stop=(eb == EB - 1),
                    )
                nc.scalar.activation(
                    hT_sb[:, hb, ms], ps[:], act, bias=b1_sb[:, hb : hb + 1]
                )

            # ---- matmul 1 + GELU ----
            # Head h-blocks run m-half-major so the PE's early xT demand
            # rate is halved while the DMA queue ramps.
            for mh in range(MH):
                for hb in range(HEAD):
                    mm1_group(w1_head[:, hb], hb, mh)
            for g0 in range(HEAD, HB, W1G):
                gn = min(W1G, HB - g0)
                w1_t = w1pool.tile([P, W1G, EB, P], mmdt, name="w1g")
                nc.sync.dma_start(
                    out=w1_t[:, 0:gn], in_=w1_v[:, g0 : g0 + gn]
                )
                for j in range(gn):
                    for mh in range(MH):
                        mm1_group(w1_t[:, j], g0 + j, mh)

            # ---- matmul 2 + bias ----
            for e2b in range(E2B):
                w2_t = w2pool.tile([P, HB, P], mmdt, name="w2t")
                nc.sync.dma_start(out=w2_t[:], in_=w2_d[e2b])
                out_sb = opool.tile([P, M], F32, name="outsb")
                for mh in range(MH):
                    ms = slice(mh * MS, (mh + 1) * MS)
                    ps2 = psum2.tile([P, MS], F32, name="ps2")
                    for hb in range(HB):
                        nc.tensor.matmul(
                            ps2[:],
                            lhsT=w2_t[:, hb, :],
                            rhs=hT_sb[:, hb, ms],
                            start=(hb == 0),
                            stop=(hb == HB - 1),
                        )
                    nc.scalar.activation(
                        out_sb[:, ms], ps2[:], IDENT, bias=b2_sb[:, e2b : e2b + 1]
                    )
                    nc.sync.dma_start(out=out_d[e2b, :, ms], in_=out_sb[:, ms])

    nc.compile()
    return nc


def pack_inputs(x, w1, b1, w2, b2):
    """Host-side shard + pack. Returns per-core input maps."""
    M_TOT = x.shape[0] * x.shape[1]
    E = x.shape[2]
    H = w1.shape[1]
    E2 = w2.shape[1]
    MC = M_TOT // N_CORES
    xf = np.ascontiguousarray(x.reshape(M_TOT, E))

    w1p = np.ascontiguousarray(
        w1.reshape(E // P, P, H // P, P).transpose(2, 1, 0, 3)
    ).astype(NP_BF16)
    w2p = np.ascontiguousarray(
        w2.reshape(H // P, P, E2 // P, P).transpose(2, 1, 0, 3)
    ).astype(NP_BF16)
    b1p = np.ascontiguousarray(b1.reshape(H // P, P).T)
    b2p = np.ascontiguousarray(b2.reshape(E2 // P, P).T)

    in_maps = []
    for i in range(N_CORES):
        xTi = np.ascontiguousarray(xf[i * MC : (i + 1) * MC].T).astype(NP_BF16)
        in_maps.append(
            {"xT": xTi, "w1p": w1p, "b1p": b1p, "w2p": w2p, "b2p": b2p}
        )
    return in_maps


def unpack_outputs(results, batch_shape=(4, 2048), E2=1024):
    M_TOT = batch_shape[0] * batch_shape[1]
    MC = M_TOT // N_CORES
    out = np.empty((M_TOT, E2), dtype=np.float32)
    for i in range(N_CORES):
        o = results[i]["outT"]  # [E2B, P, MC]
        out[i * MC : (i + 1) * MC] = o.transpose(2, 0, 1).reshape(MC, E2)
    return out.reshape(*batch_shape, E2)


_NC_CACHE = {}


def _get_nc():
    if "nc" not in _NC_CACHE:
        _NC_CACHE["nc"] = build_nc()
    return _NC_CACHE["nc"]


def kernel(x, w1, b1, w2, b2):
    nc = _get_nc()
    in_maps = pack_inputs(
        np.asarray(x, dtype=np.float32),
        np.asarray(w1, dtype=np.float32),
        np.asarray(b1, dtype=np.float32),
        np.asarray(w2, dtype=np.float32),
        np.asarray(b2, dtype=np.float32),
    )
    res = run_bass_kernel_spmd(nc, in_maps, core_ids=list(range(N_CORES))).results
    return unpack_outputs(res, batch_shape=(x.shape[0], x.shape[1]), E2=w2.shape[1])


# revision 11
# speedup vs baseline: 1.0130x; 1.0083x over previous
"""Trainium2 Bass kernel for CustomMLP: out = GELU(x@W1+b1)@W2 + b2.

x: (4, 2048, 1024) f32, W1: (1024, 4096), b1: (4096,), W2: (4096, 1024),
b2: (1024,). Data-parallel over the 8192 flattened rows: each of the 8
NeuronCores handles 1024 rows with fully replicated weights (no
collectives).

Per-core layout (everything transposed so both matmuls contract on the
partition axis with no on-chip transposes):
  xT   [1024(e), 1024(m)]           = x_shard^T, bf16
  hT   [h, m] computed on chip      (GELU applied on PSUM eviction), bf16
  outT [1024(e2), 1024(m)]          f32; host transposes back

matmul1: psum[h_blk, m] += w1[e_blk, h_blk].T @ xT[e_blk, m]
matmul2: psum[e2_blk, m] += w2[h_blk, e2_blk].T @ hT[h_blk, m]

All matmul operands are bf16 (fp32 PSUM accumulation): same 1 cycle/row
PE rate as fp32r but FWL (fast weight load) hides LDWEIGHTS, and HBM
traffic halves.  Weights are host-packed so every DMA lands contiguous
per partition.  DMA is split across both HWDGE queues: w1 on the scalar
queue (batched 4 h-blocks per DMA), xT/w2/out on the sync queue.  A few
warm-up matmuls on memset tiles run during the head DMA wait so the HAM
clock-gate ramp (1.2->2.4 GHz) is absorbed before real work arrives.
"""
import ml_dtypes
import numpy as np

import concourse.bass as bass
import concourse.mybir as mybir
import concourse.tile as tile
from concourse import bacc
from concourse.bass_utils import run_bass_kernel_spmd

P = 128
N_CORES = 8

F32 = mybir.dt.float32
BF16 = mybir.dt.bfloat16
NP_BF16 = ml_dtypes.bfloat16
GELU = mybir.ActivationFunctionType.Gelu
IDENT = mybir.ActivationFunctionType.Identity


def build_nc(M=1024, E=1024, H=4096, E2=1024, mm_dtype=BF16, act=GELU, nwarm=9):
    """Build + compile the per-core program. M/E/H/E2 parameterized so a
    scaled-down version can run in CoreSim."""
    EB, HB, E2B = E // P, H // P, E2 // P
    MH = max(1, M // 512)  # m halves (moving-dim chunks of <=512)
    MS = M // MH           # moving chunk size
    HEAD = min(4, HB)      # singly-loaded head h-blocks
    W1G = min(4, HB - HEAD) or 1  # w1 h-blocks per grouped DMA

    mmdt = mm_dtype
    nc = bacc.Bacc(None, target_bir_lowering=False)
    xT_d = nc.declare_dram_parameter("xT", [E, M], mmdt, isOutput=False)
    w1_d = nc.declare_dram_parameter("w1p", [HB, P, EB, P], mmdt, isOutput=False)
    b1_d = nc.declare_dram_parameter("b1p", [P, HB], F32, isOutput=False)
    w2_d = nc.declare_dram_parameter("w2p", [E2B, P, HB, P], mmdt, isOutput=False)
    b2_d = nc.declare_dram_parameter("b2p", [P, E2B], F32, isOutput=False)
    out_d = nc.declare_dram_parameter("outT", [E2B, P, M], F32, isOutput=True)

    xT_v = xT_d.rearrange("(eb p) m -> p eb m", p=P)
    w1_v = w1_d.rearrange("hb p eb q -> p hb eb q")

    with tile.TileContext(nc) as tc:
        with (
            tc.tile_pool(name="const", bufs=1) as cpool,
            tc.tile_pool(name="xp", bufs=1) as xpool,
            tc.tile_pool(name="hp", bufs=1) as hpool,
            tc.tile_pool(name="w1h", bufs=1) as w1head,
            tc.tile_pool(name="w1p", bufs=3) as w1pool,
            tc.tile_pool(name="w2p", bufs=3) as w2pool,
            tc.tile_pool(name="op", bufs=2) as opool,
            tc.tile_pool(name="ps1", bufs=3, space="PSUM") as psum1,
            tc.tile_pool(name="ps2", bufs=3, space="PSUM") as psum2,
            tc.tile_pool(name="psw", bufs=1, space="PSUM") as psumw,
        ):
            # ---- HAM warm-up: dummy matmuls on memset tiles keep the PE
            # busy through the clock-gate ramp while the head DMAs land.
            if nwarm:
                wm_l = cpool.tile([P, P], mmdt, name="wml")
                wm_r = cpool.tile([P, MS], mmdt, name="wmr")
                nc.vector.memset(wm_l[:], 0.0)
                nc.vector.memset(wm_r[:], 0.0)
                ps_w = psumw.tile([P, MS], F32, name="psw")
                for _ in range(nwarm):
                    nc.tensor.matmul(
                        ps_w[:], lhsT=wm_l[:], rhs=wm_r[:], start=True, stop=True
                    )

            # ---- head DMAs, all on the sync HWDGE queue in consumption
            # order: w1 hb0 first, then xT eb0 so matmul 0 starts early,
            # then the bulk loads.
            # Interleave ~256KB xT chunks with single w1 head blocks so
            # early PE demand (eb-chunk pacing in group 0, then one w1
            # block per 1.7us group) tracks the ~358 GB/s supply with
            # fine-grained completion semaphores.
            w1_head = w1head.tile([P, HEAD, EB, P], mmdt, name="w1t")
            xT_sb = xpool.tile([P, EB, M], mmdt, name="xT")
            nc.sync.dma_start(out=w1_head[:, 0], in_=w1_d[0])
            XC = 2  # xT eb-blocks per head DMA
            xt_chunks = [
                (c0, min(c0 + XC, EB)) for c0 in range(0, EB, XC)
            ]
            nhead = max(len(xt_chunks), HEAD - 1)
            for k in range(nhead):
                if k < len(xt_chunks):
                    c0, c1 = xt_chunks[k]
                    nc.sync.dma_start(
                        out=xT_sb[:, c0:c1, 0:MS], in_=xT_v[:, c0:c1, 0:MS]
                    )
                if 1 + k < HEAD:
                    nc.sync.dma_start(
                        out=w1_head[:, 1 + k], in_=w1_d[1 + k]
                    )
            b1_sb = cpool.tile([P, HB], F32, name="b1s")
            b2_sb = cpool.tile([P, E2B], F32, name="b2s")
            nc.sync.dma_start(out=b1_sb[:], in_=b1_d[:])
            nc.sync.dma_start(out=b2_sb[:], in_=b2_d[:])
            for mh in range(1, MH):
                ms = slice(mh * MS, (mh + 1) * MS)
                half = EB // 2 or EB
                for c0 in range(0, EB, half):
                    c1 = min(c0 + half, EB)
                    nc.sync.dma_start(
                        out=xT_sb[:, c0:c1, ms], in_=xT_v[:, c0:c1, ms]
                    )

            hT_sb = hpool.tile([P, HB, M], mmdt, name="hT")

            def mm1_group(w1_t, hb, mh):
                ms = slice(mh * MS, (mh + 1) * MS)
                ps = psum1.tile([P, MS], F32, name="ps1")
                for eb in range(EB):
                    nc.tensor.matmul(
                        ps[:],
                        lhsT=w1_t[:, eb, :],
                        rhs=xT_sb[:, eb, ms],
                        start=(eb == 0),
                        stop=(eb == EB - 1),
                    )
                nc.scalar.activation(
                    hT_sb[:, hb, ms], ps[:], act, bias=b1_sb[:, hb : hb + 1]
                )

            # ---- matmul 1 + GELU ----
            # Head h-blocks run m-half-major so the PE's early xT demand
            # rate is halved while the DMA queue ramps.
            for mh in range(MH):
                for hb in range(HEAD):
                    mm1_group(w1_head[:, hb], hb, mh)
            for g0 in range(HEAD, HB, W1G):
                gn = min(W1G, HB - g0)
                w1_t = w1pool.tile([P, W1G, EB, P], mmdt, name="w1g")
                nc.sync.dma_start(
                    out=w1_t[:, 0:gn], in_=w1_v[:, g0 : g0 + gn]
                )
                for j in range(gn):
                    for mh in range(MH):
                        mm1_group(w1_t[:, j], g0 + j, mh)

            # ---- matmul 2 + bias ----
            for e2b in range(E2B):
                w2_t = w2pool.tile([P, HB, P], mmdt, name="w2t")
                nc.sync.dma_start(out=w2_t[:], in_=w2_d[e2b])
                out_sb = opool.tile([P, M], F32, name="outsb")
                for mh in range(MH):
                    ms = slice(mh * MS, (mh + 1) * MS)
                    ps2 = psum2.tile([P, MS], F32, name="ps2")
                    for hb in range(HB):
                        nc.tensor.matmul(
                            ps2[:],
                            lhsT=w2_t[:, hb, :],
                            rhs=hT_sb[:, hb, ms],
                            start=(hb == 0),
                            stop=(hb == HB - 1),
                        )
                    nc.scalar.activation(
                        out_sb[:, ms], ps2[:], IDENT, bias=b2_sb[:, e2b : e2b + 1]
                    )
                    nc.sync.dma_start(out=out_d[e2b, :, ms], in_=out_sb[:, ms])

    nc.compile()
    return nc


def pack_inputs(x, w1, b1, w2, b2):
    """Host-side shard + pack. Returns per-core input maps."""
    M_TOT = x.shape[0] * x.shape[1]
    E = x.shape[2]
    H = w1.shape[1]
    E2 = w2.shape[1]
    MC = M_TOT // N_CORES
    xf = np.ascontiguousarray(x.reshape(M_TOT, E))

    w1p = np.ascontiguousarray(
        w1.reshape(E // P, P, H // P, P).transpose(2, 1, 0, 3)
    ).astype(NP_BF16)
    w2p = np.ascontiguousarray(
        w2.reshape(H // P, P, E2 // P, P).transpose(2, 1, 0, 3)
    ).astype(NP_BF16)
    b1p = np.ascontiguousarray(b1.reshape(H // P, P).T)
    b2p = np.ascontiguousarray(b2.reshape(E2 // P, P).T)

    in_maps = []
    for i in range(N_CORES):
        xTi = np.ascontiguousarray(xf[i * MC : (i + 1) * MC].T).astype(NP_BF16)
        in_maps.append(
            {"xT": xTi, "w1p": w1p, "b1p": b1p, "w2p": w2p, "b2p": b2p}
        )
    return in_maps


def unpack_outputs(results, batch_shape=(4, 2048), E2=1024):
    M_TOT = batch_shape[0] * batch_shape[1]
    MC = M_TOT // N_CORES
    out = np.empty((M_TOT, E2), dtype=np.float32)
    for i in range(N_CORES):
        o = results[i]["outT"]  # [E2B, P, MC]
        out[i * MC : (i + 1) * MC] = o.transpose(2, 0, 1).reshape(MC, E2)
    return out.reshape(*batch_shape, E2)


_NC_CACHE = {}


def _get_nc():
    if "nc" not in _NC_CACHE:
        _NC_CACHE["nc"] = build_nc()
    return _NC_CACHE["nc"]


def kernel(x, w1, b1, w2, b2):
    nc = _get_nc()
    in_maps = pack_inputs(
        np.asarray(x, dtype=np.float32),
        np.asarray(w1, dtype=np.float32),
        np.asarray(b1, dtype=np.float32),
        np.asarray(w2, dtype=np.float32),
        np.asarray(b2, dtype=np.float32),
    )
    res = run_bass_kernel_spmd(nc, in_maps, core_ids=list(range(N_CORES))).results
    return unpack_outputs(res, batch_shape=(x.shape[0], x.shape[1]), E2=w2.shape[1])


# revision 26
# speedup vs baseline: 1.0145x; 1.0014x over previous
"""Trainium2 Bass kernel for CustomMLP: out = GELU(x@W1+b1)@W2 + b2.

x: (4, 2048, 1024) f32, W1: (1024, 4096), b1: (4096,), W2: (4096, 1024),
b2: (1024,). Data-parallel over the 8192 flattened rows: each of the 8
NeuronCores handles 1024 rows with fully replicated weights (no
collectives).

Per-core layout (everything transposed so both matmuls contract on the
partition axis with no on-chip transposes):
  xT   [1024(e), 1024(m)]           = x_shard^T, bf16
  hT   [h, m] computed on chip      (GELU applied on PSUM eviction), bf16
  outT [1024(e2), 1024(m)]          bf16; host transposes back + upcasts

matmul1: psum[h_blk, m] += w1[e_blk, h_blk].T @ xT[e_blk, m]
matmul2: psum[e2_blk, m] += w2[h_blk, e2_blk].T @ hT[h_blk, m]

All matmul operands are bf16 (fp32 PSUM accumulation): same 1 cycle/row
PE rate as fp32r, but FWL (fast weight load) hides LDWEIGHTS and HBM
traffic halves (rel err ~4e-3, well under the 2e-2 gate).  Weights are
host-packed so every DMA lands contiguous per partition.  All DMAs ride
the sync HWDGE queue in consumption order, batched so descriptor-issue
overhead (~0.6us each) stays off the critical path while completion
granularity at the head (~256KB) matches the PE's early demand.  A few
warm-up matmuls on memset tiles run during the head DMA wait so the HAM
clock-gate ramp (1.2->2.4 GHz) is absorbed before real data arrives;
the last output block is evicted in 256-col chunks to shorten the
serial ACT->DMA tail.
"""
import ml_dtypes
import numpy as np

import concourse.bass as bass
import concourse.mybir as mybir
import concourse.tile as tile
from concourse import bacc
from concourse.bass_utils import run_bass_kernel_spmd

P = 128
N_CORES = 8

F32 = mybir.dt.float32
BF16 = mybir.dt.bfloat16
NP_BF16 = ml_dtypes.bfloat16
GELU = mybir.ActivationFunctionType.Gelu
IDENT = mybir.ActivationFunctionType.Identity


def build_nc(
    M=1024, E=1024, H=4096, E2=1024, mm_dtype=BF16, act=GELU, nwarm=9, share_lw=1
):
    """Build + compile the per-core program. M/E/H/E2 parameterized so a
    scaled-down version can run in CoreSim."""
    EB, HB, E2B = E // P, H // P, E2 // P
    MH = max(1, M // 512)  # m halves (moving-dim chunks of <=512)
    MS = M // MH           # moving chunk size
    HEAD = min(4, HB)      # singly-loaded head h-blocks
    W1G = min(4, HB - HEAD) or 1  # w1 h-blocks per grouped DMA

    mmdt = mm_dtype
    nc = bacc.Bacc(None, target_bir_lowering=False)
    xT_d = nc.declare_dram_parameter("xT", [E, M], mmdt, isOutput=False)
    w1_d = nc.declare_dram_parameter("w1p", [HB, P, EB, P], mmdt, isOutput=False)
    b1_d = nc.declare_dram_parameter("b1p", [P, HB], F32, isOutput=False)
    w2_d = nc.declare_dram_parameter("w2p", [E2B, P, HB, P], mmdt, isOutput=False)
    b2_d = nc.declare_dram_parameter("b2p", [P, E2B], F32, isOutput=False)
    out_d = nc.declare_dram_parameter("outT", [E2B, P, M], BF16, isOutput=True)

    xT_v = xT_d.rearrange("(eb p) m -> p eb m", p=P)
    w1_v = w1_d.rearrange("hb p eb q -> p hb eb q")

    with tile.TileContext(nc) as tc:
        with (
            tc.tile_pool(name="const", bufs=1) as cpool,
            tc.tile_pool(name="xp", bufs=1) as xpool,
            tc.tile_pool(name="hp", bufs=1) as hpool,
            tc.tile_pool(name="w1h", bufs=1) as w1head,
            tc.tile_pool(name="w1p", bufs=3) as w1pool,
            tc.tile_pool(name="w2p", bufs=3) as w2pool,
            tc.tile_pool(name="op", bufs=2) as opool,
            tc.tile_pool(name="ps1", bufs=4, space="PSUM") as psum1,
            tc.tile_pool(name="ps2", bufs=4, space="PSUM") as psum2,
        ):
            # ---- HAM warm-up: dummy matmuls on memset tiles keep the PE
            # busy through the clock-gate ramp while the head DMAs land.
            if nwarm:
                wm_l = cpool.tile([P, P], mmdt, name="wml")
                wm_r = cpool.tile([P, MS], mmdt, name="wmr")
                nc.vector.memset(wm_l[:], 0.0)
                nc.vector.memset(wm_r[:], 0.0)
                ps_w = psum1.tile([P, MS], F32, name="ps1")
                for _ in range(nwarm):
                    nc.tensor.matmul(
                        ps_w[:], lhsT=wm_l[:], rhs=wm_r[:], start=True, stop=True
                    )

            # ---- head DMAs, all on the sync HWDGE queue in consumption
            # order: w1 hb0 first, then xT eb0 so matmul 0 starts early,
            # then the bulk loads.
            # Interleave ~256KB xT chunks with single w1 head blocks so
            # early PE demand (eb-chunk pacing in group 0, then one w1
            # block per 1.7us group) tracks the ~358 GB/s supply with
            # fine-grained completion semaphores.
            w1_head = w1head.tile([P, HEAD, EB, P], mmdt, name="w1t")
            xT_sb = xpool.tile([P, EB, M], mmdt, name="xT")
            nc.sync.dma_start(out=w1_head[:, 0], in_=w1_d[0])
            XC = 2  # xT eb-blocks per head DMA
            xt_chunks = [
                (c0, min(c0 + XC, EB)) for c0 in range(0, EB, XC)
            ]
            nhead = max(len(xt_chunks), HEAD - 1)
            for k in range(nhead):
                if k < len(xt_chunks):
                    c0, c1 = xt_chunks[k]
                    nc.sync.dma_start(
                        out=xT_sb[:, c0:c1, 0:MS], in_=xT_v[:, c0:c1, 0:MS]
                    )
                if 1 + k < HEAD:
                    nc.sync.dma_start(
                        out=w1_head[:, 1 + k], in_=w1_d[1 + k]
                    )
            b1_sb = cpool.tile([P, HB], F32, name="b1s")
            b2_sb = cpool.tile([P, E2B], F32, name="b2s")
            nc.sync.dma_start(out=b1_sb[:], in_=b1_d[:])
            nc.sync.dma_start(out=b2_sb[:], in_=b2_d[:])
            for mh in range(1, MH):
                ms = slice(mh * MS, (mh + 1) * MS)
                half = EB // 2 or EB
                for c0 in range(0, EB, half):
                    c1 = min(c0 + half, EB)
                    nc.sync.dma_start(
                        out=xT_sb[:, c0:c1, ms], in_=xT_v[:, c0:c1, ms]
                    )

            hT_sb = hpool.tile([P, HB, M], mmdt, name="hT")

            def mm1_group(w1_t, hb, mh):
                ms = slice(mh * MS, (mh + 1) * MS)
                ps = psum1.tile([P, MS], F32, name="ps1")
                for eb in range(EB):
                    nc.tensor.matmul(
                        ps[:],
                        lhsT=w1_t[:, eb, :],
                        rhs=xT_sb[:, eb, ms],
                        start=(eb == 0),
                        stop=(eb == EB - 1),
                    )
                nc.scalar.activation(
                    hT_sb[:, hb, ms], ps[:], act, bias=b1_sb[:, hb : hb + 1]
                )

            # ---- matmul 1 + GELU ----
            # Head h-blocks run m-half-major so the PE's early xT demand
            # rate is halved while the DMA queue ramps.
            for mh in range(MH):
                for hb in range(HEAD):
                    mm1_group(w1_head[:, hb], hb, mh)
            # Tail h-blocks stream both m-halves back-to-back under the
            # same stationary weight block (consecutive matmuls share
            # lhsT, halving the LDWEIGHTS/instruction stream).
            for g0 in range(HEAD, HB, W1G):
                gn = min(W1G, HB - g0)
                w1_t = w1pool.tile([P, W1G, EB, P], mmdt, name="w1g")
                nc.sync.dma_start(
                    out=w1_t[:, 0:gn], in_=w1_v[:, g0 : g0 + gn]
                )
                for j in range(gn):
                    pss = [psum1.tile([P, MS], F32, name="ps1") for _ in range(MH)]
                    for eb in range(EB):
                        for mh in range(MH):
                            r = nc.tensor.matmul(
                                pss[mh][:],
                                lhsT=w1_t[:, j, eb, :],
                                rhs=xT_sb[:, eb, mh * MS : (mh + 1) * MS],
                                start=(eb == 0),
                                stop=(eb == EB - 1),
                            )
                            if share_lw and mh > 0:
                                r.ins.ldweights = False
                    for mh in range(MH):
                        nc.scalar.activation(
                            hT_sb[:, g0 + j, mh * MS : (mh + 1) * MS],
                            pss[mh][:],
                            act,
                            bias=b1_sb[:, g0 + j : g0 + j + 1],
                        )

            # ---- matmul 2 + bias ----
            for e2b in range(E2B):
                w2_t = w2pool.tile([P, HB, P], mmdt, name="w2t")
                nc.sync.dma_start(out=w2_t[:], in_=w2_d[e2b])
                out_sb = opool.tile([P, M], BF16, name="outsb")
                # Finer chunks on the last block shorten the serial
                # ACT->DMA tail after the final matmul.  Chunks are the
                # inner loop so consecutive matmuls share lhsT.
                if e2b < E2B - 1:
                    csz = MS
                    chunks = list(range(0, M, csz))
                    pss = [psum2.tile([P, csz], F32, name="ps2") for _ in chunks]
                    for hb in range(HB):
                        for ci, c0 in enumerate(chunks):
                            r = nc.tensor.matmul(
                                pss[ci][:],
                                lhsT=w2_t[:, hb, :],
                                rhs=hT_sb[:, hb, c0 : c0 + csz],
                                start=(hb == 0),
                                stop=(hb == HB - 1),
                            )
                            if share_lw and ci > 0:
                                r.ins.ldweights = False
                    for ci, c0 in enumerate(chunks):
                        cs = slice(c0, c0 + csz)
                        nc.scalar.activation(
                            out_sb[:, cs],
                            pss[ci][:],
                            IDENT,
                            bias=b2_sb[:, e2b : e2b + 1],
                        )
                        nc.sync.dma_start(out=out_d[e2b, :, cs], in_=out_sb[:, cs])
                else:
                    # chunk-outer on the last block: each chunk's ACT+store
                    # overlaps the next chunk's matmuls, so only the final
                    # small chunk's eviction sits on the critical tail.
                    csz = max(MS // 2, 128)
                    for c0 in range(0, M, csz):
                        cs = slice(c0, c0 + csz)
                        ps2 = psum2.tile([P, csz], F32, name="ps2")
                        for hb in range(HB):
                            nc.tensor.matmul(
                                ps2[:],
                                lhsT=w2_t[:, hb, :],
                                rhs=hT_sb[:, hb, cs],
                                start=(hb == 0),
                                stop=(hb == HB - 1),
                            )
                        nc.scalar.activation(
                            out_sb[:, cs], ps2[:], IDENT, bias=b2_sb[:, e2b : e2b + 1]
                        )
                        nc.sync.dma_start(out=out_d[e2b, :, cs], in_=out_sb[:, cs])

    nc.compile()
    return nc


def pack_inputs(x, w1, b1, w2, b2):
    """Host-side shard + pack. Returns per-core input maps."""
    M_TOT = x.shape[0] * x.shape[1]
    E = x.shape[2]
    H = w1.shape[1]
    E2 = w2.shape[1]
    MC = M_TOT // N_CORES
    xf = np.ascontiguousarray(x.reshape(M_TOT, E))

    w1p = np.ascontiguousarray(
        w1.reshape(E // P, P, H // P, P).transpose(2, 1, 0, 3)
    ).astype(NP_BF16)
    w2p = np.ascontiguousarray(
        w2.reshape(H // P, P, E2 // P, P).transpose(2, 1, 0, 3)
    ).astype(NP_BF16)
    b1p = np.ascontiguousarray(b1.reshape(H // P, P).T)
    b2p = np.ascontiguousarray(b2.reshape(E2 // P, P).T)

    in_maps = []
    for i in range(N_CORES):
        xTi = np.ascontiguousarray(xf[i * MC : (i + 1) * MC].T).astype(NP_BF16)
        in_maps.append(
            {"xT": xTi, "w1p": w1p, "b1p": b1p, "w2p": w2p, "b2p": b2p}
        )
    return in_maps


def unpack_outputs(results, batch_shape=(4, 2048), E2=1024):
    M_TOT = batch_shape[0] * batch_shape[1]
    MC = M_TOT // N_CORES
    out = np.empty((M_TOT, E2), dtype=np.float32)
    for i in range(N_CORES):
        o = np.asarray(results[i]["outT"]).astype(np.float32)  # [E2B, P, MC]
        out[i * MC : (i + 1) * MC] = o.transpose(2, 0, 1).reshape(MC, E2)
    return out.reshape(*batch_shape, E2)


_NC_CACHE = {}


def _get_nc():
    if "nc" not in _NC_CACHE:
        _NC_CACHE["nc"] = build_nc()
    return _NC_CACHE["nc"]


def kernel(x, w1, b1, w2, b2):
    nc = _get_nc()
    in_maps = pack_inputs(
        np.asarray(x, dtype=np.float32),
        np.asarray(w1, dtype=np.float32),
        np.asarray(b1, dtype=np.float32),
        np.asarray(w2, dtype=np.float32),
        np.asarray(b2, dtype=np.float32),
    )
    res = run_bass_kernel_spmd(nc, in_maps, core_ids=list(range(N_CORES))).results
    return unpack_outputs(res, batch_shape=(x.shape[0], x.shape[1]), E2=w2.shape[1])
